# revision 12
# baseline (speedup 1.0000x reference)
"""Bilateral grid slice+apply on 8 Trainium2 NeuronCores.

Gather-free formulation: the per-pixel trilinear interpolation is expressed
in the hat-function basis  hat(a) = relu(1 - |a|)  and evaluated densely as
matmuls with the (tiny) grid as the stationary operand:

    coeffs[n, z, c] = sum_{y,x} hy(n,y) hx(n,x) * G[y, x, z, c]     (PE, K=256)
    out[n, c3]      = sum_{z,j} hz(n,z) * xt(n,j) * coeffs[n, z, 4c3+j]

Pixels ride the matmul free dimension.  v4 layout: the PE only runs the
irreducible matmuls (hat-args, the two K=128 mains, the K=96 reduce); all
hat *replications* (hy/hx/hz fan-out to the 128/96-row product layouts) are
stride-0 SBUF->SBUF DMAs, and the xt fan-out is a stride-0 DMA straight
from DRAM.  Tiles are processed in pairs (free dim 2F=1024) to halve
instruction dispatch and DMA count; matmuls still run at N=512 per PSUM
bank.  bf16 everywhere except the coordinate path (fp32r) and PSUM (fp32).

Data parallel: pixels are sharded across the 8 cores; the 16x16x8x12 grid
is replicated (host bakes it into the stationary operands).
"""
import ml_dtypes
import numpy as np
from contextlib import ExitStack

import concourse.bass as bass
import concourse.bacc as bacc
import concourse.mybir as mybir
from concourse import tile
from concourse.bass_utils import run_bass_kernel_spmd

F = 512             # pixels per matmul pass (one fp32 PSUM bank)
NCORES = 8
B, H, W = 4, 1080, 1920
NTOT = B * H * W                  # 8294400
NPC = NTOT // NCORES              # 1036800 per core
T = NPC // F                      # 2025 tiles per core
LUM = (0.2126, 0.7152, 0.0722)

_CACHE = {}
BF16 = ml_dtypes.bfloat16


def _make_stationaries(grid):
    g = grid.astype(np.float32)
    stP0 = np.zeros((5, 72), np.float32)      # rows (r,g,b,cx,cy)
    for m in range(16):
        stP0[4, m] = 15.0                     # gy from cy
    for m in range(32, 48):
        stP0[3, m] = 15.0                     # gx from cx
    for m in range(64, 72):
        stP0[0, m] = 7.0 * LUM[0]
        stP0[1, m] = 7.0 * LUM[1]
        stP0[2, m] = 7.0 * LUM[2]
    bias40 = np.zeros((72, 1), np.float32)
    bias40[:16, 0] = -np.arange(16)
    bias40[32:48, 0] = -np.arange(16)
    bias40[64:72, 0] = -np.arange(8)

    stHXa = np.zeros((16, 128), np.float32)
    for m in range(128):
        stHXa[m % 16, m] = 1.0

    stMAIN = np.zeros((2, 128, 96), np.float32)
    for p in range(2):
        for k in range(128):
            stMAIN[p, k, :] = g[p * 8 + k // 16, k % 16].reshape(96)

    stRED = np.zeros((96, 3), np.float32)
    for z in range(8):
        for c3 in range(3):
            for j in range(4):
                stRED[z * 12 + c3 * 4 + j, c3] = 1.0

    return dict(stP0=stP0, bias40=bias40, stHXa=stHXa,
                stMAINa=stMAIN[0], stMAINb=stMAIN[1], stRED=stRED)


def _cast_stationaries(stats):
    """bf16 for everything that feeds a matmul; fp32 for the Act bias and
    the fp32r coordinate path."""
    keep = ("bias40", "stP0")
    return {k: (v if k in keep else v.astype(BF16)) for k, v in stats.items()}


def make_in_map(p, c, grid):
    """One core's input map from flat p [n,3], c [n,2] and the grid."""
    n = p.shape[0]
    stats = _cast_stationaries(_make_stationaries(np.asarray(grid, np.float32)))
    in5 = np.ascontiguousarray(
        np.stack([p[:, 0], p[:, 1], p[:, 2], c[:, 0], c[:, 1]]))
    inx = np.ascontiguousarray(
        np.stack([p[:, 0], p[:, 1], p[:, 2],
                  np.ones(n, np.float32)])).astype(BF16)
    return {"in5": in5, "inx": inx, **stats}


def build_kernel(ntiles=T, num_cores=NCORES, reps=1):
    nc = bacc.Bacc("TRN2", target_bir_lowering=False, debug=False,
                   num_devices=num_cores)
    NP = ntiles * F
    f32 = mybir.dt.float32
    f32r = mybir.dt.float32r
    bf16 = mybir.dt.bfloat16

    in5 = nc.declare_dram_parameter("in5", [5, NP], f32r, isOutput=False)
    inx = nc.declare_dram_parameter("inx", [4, NP], bf16, isOutput=False)
    decls = {}
    for nm, shp, dt_ in (("stP0", [5, 72], f32r), ("bias40", [72, 1], f32),
                         ("stHXa", [16, 128], bf16),
                         ("stMAINa", [128, 96], bf16),
                         ("stMAINb", [128, 96], bf16),
                         ("stRED", [96, 3], bf16)):
        decls[nm] = nc.declare_dram_parameter(nm, shp, dt_, isOutput=False)
    out3 = nc.declare_dram_parameter("out3", [3, NP], f32, isOutput=True)

    P2 = 2 * F
    npairs, tail = ntiles // 2, ntiles % 2

    with tile.TileContext(nc) as tc:
        with ExitStack() as ctx:
            stp = ctx.enter_context(tc.tile_pool(name="stats", bufs=1))
            sP0 = stp.tile([5, 72], f32r, tag="sP0")
            sB40 = stp.tile([72, 1], f32, tag="sB40")
            sHXa_t = stp.tile([48, 128], bf16, tag="sHXa")
            sHXa = sHXa_t[32:48, :]
            sMa = stp.tile([128, 96], bf16, tag="sMa")
            sMb = stp.tile([128, 96], bf16, tag="sMb")
            sRED = stp.tile([96, 3], bf16, tag="sRED")
            for t_, nm in ((sP0[:], "stP0"), (sB40[:], "bias40"),
                           (sHXa, "stHXa"),
                           (sMa[:], "stMAINa"), (sMb[:], "stMAINb"),
                           (sRED[:], "stRED")):
                nc.sync.dma_start(t_, decls[nm].ap())

            sb_in = ctx.enter_context(tc.tile_pool(name="sb_in", bufs=3))
            sb_mid = ctx.enter_context(tc.tile_pool(name="sb_mid", bufs=3))
            sb_rep = ctx.enter_context(tc.tile_pool(name="sb_rep", bufs=2))
            sb_w = ctx.enter_context(tc.tile_pool(name="sb_w", bufs=2))
            ps_args = ctx.enter_context(
                tc.tile_pool(name="ps_args", bufs=1, space="PSUM"))
            ps_hx = ctx.enter_context(
                tc.tile_pool(name="ps_hx", bufs=1, space="PSUM"))
            ps_cf = ctx.enter_context(
                tc.tile_pool(name="ps_cf", bufs=1, space="PSUM"))
            ps_out = ctx.enter_context(
                tc.tile_pool(name="ps_out", bufs=1, space="PSUM"))

            def mm(out, lhsT, rhs, start=True, stop=True):
                nc.tensor.matmul(out, lhsT, rhs, start=start, stop=stop)

            def pair_body(cols, c0):
                """Emit one pair (or tail) body: `cols` pixels starting at
                full-row column c0."""
                IN5 = sb_in.tile([5, P2], f32r, tag="in5p", name="IN5")
                nc.gpsimd.dma_start(IN5[:, 0:cols],
                                    in5.ap()[:, c0:c0 + cols])
                X96 = sb_in.tile([96, P2], bf16, tag="x96p", name="X96")
                nc.gpsimd.dma_start(
                    X96[:, 0:cols],
                    inx.ap()[:, c0:c0 + cols].partition_broadcast(24))

                argsP = ps_args.tile([72, P2], f32, tag="args", name="argsP")
                for h in range(0, cols, F):
                    mm(argsP[:, h:h + F], sP0[:], IN5[:, h:h + F])
                tabs = sb_mid.tile([72, P2], bf16, tag="tabs", name="tabs")
                nc.scalar.activation(tabs[:, 0:cols], argsP[:, 0:cols],
                                     mybir.ActivationFunctionType.Abs,
                                     bias=sB40[:], scale=1.0)
                hats = sb_mid.tile([72, P2], bf16, tag="hats", name="hats")
                nc.scalar.activation(hats[:, 0:cols], tabs[:, 0:cols],
                                     mybir.ActivationFunctionType.Relu,
                                     bias=1.0, scale=-1.0)

                # hat replications: stride-0 SBUF->SBUF DMAs
                HYa = sb_rep.tile([128, P2], bf16, tag="hya", name="HYa")
                HYb = sb_rep.tile([128, P2], bf16, tag="hyb", name="HYb")
                HZ = sb_rep.tile([96, P2], bf16, tag="hz", name="HZ")
                nc.sync.dma_start(
                    HYa[:, 0:cols],
                    hats[0:8, 0:cols].unsqueeze(1).broadcast_to(
                        (8, 16, cols)))
                nc.sync.dma_start(
                    HYb[:, 0:cols],
                    hats[8:16, 0:cols].unsqueeze(1).broadcast_to(
                        (8, 16, cols)))
                nc.sync.dma_start(
                    HZ[:, 0:cols],
                    hats[64:72, 0:cols].unsqueeze(1).broadcast_to(
                        (8, 12, cols)))

                HX = ps_hx.tile([128, P2], f32, tag="hx", name="HX")
                for h in range(0, cols, F):
                    mm(HX[:, h:h + F], sHXa, hats[32:48, h:h + F])
                Wa = sb_w.tile([128, P2], bf16, tag="wa", name="Wa")
                Wb = sb_w.tile([128, P2], bf16, tag="wb", name="Wb")
                HZX = sb_w.tile([96, P2], bf16, tag="hzx", name="HZX")
                nc.vector.tensor_tensor(out=Wa[:, 0:cols],
                                        in0=HYa[:, 0:cols],
                                        in1=HX[:, 0:cols],
                                        op=mybir.AluOpType.mult)
                nc.vector.tensor_tensor(out=Wb[:, 0:cols],
                                        in0=HYb[:, 0:cols],
                                        in1=HX[:, 0:cols],
                                        op=mybir.AluOpType.mult)
                nc.vector.tensor_tensor(out=HZX[:, 0:cols],
                                        in0=HZ[:, 0:cols],
                                        in1=X96[:, 0:cols],
                                        op=mybir.AluOpType.mult)

                CF = ps_cf.tile([96, P2], f32, tag="cf", name="CF")
                for h in range(0, cols, F):
                    mm(CF[:, h:h + F], sMa[:], Wa[:, h:h + F],
                       start=True, stop=False)
                    mm(CF[:, h:h + F], sMb[:], Wb[:, h:h + F],
                       start=False, stop=True)
                M2 = sb_w.tile([96, P2], bf16, tag="m2", name="M2")
                nc.vector.tensor_tensor(out=M2[:, 0:cols],
                                        in0=CF[:, 0:cols],
                                        in1=HZX[:, 0:cols],
                                        op=mybir.AluOpType.mult)
                OUT3 = ps_out.tile([3, P2], f32, tag="o3", name="OUT3")
                for h in range(0, cols, F):
                    mm(OUT3[:, h:h + F], sRED[:], M2[:, h:h + F])
                OUTS = sb_in.tile([3, P2], f32, tag="outs", name="OUTS")
                nc.scalar.copy(OUTS[:, 0:cols], OUT3[:, 0:cols])
                nc.gpsimd.dma_start(out3.ap()[:, c0:c0 + cols],
                                    OUTS[:, 0:cols])

            for _rep in range(reps):
                for pi in range(npairs):
                    pair_body(P2, pi * P2)
                if tail:
                    pair_body(F, npairs * P2)

    nc.compile()
    return nc


def kernel(pixels: np.ndarray, coords: np.ndarray, grid: np.ndarray) -> np.ndarray:
    assert pixels.shape == (B, H, W, 3) and coords.shape == (B, H, W, 2)
    p = np.asarray(pixels, np.float32).reshape(-1, 3)
    c = np.asarray(coords, np.float32).reshape(-1, 2)
    r = np.ascontiguousarray(p[:, 0]); g = np.ascontiguousarray(p[:, 1])
    b = np.ascontiguousarray(p[:, 2])
    cx = np.ascontiguousarray(c[:, 0]); cy = np.ascontiguousarray(c[:, 1])
    ones = np.ones(NPC, np.float32)

    stats = _cast_stationaries(_make_stationaries(np.asarray(grid, np.float32)))
    in_maps = []
    for cid in range(NCORES):
        s = slice(cid * NPC, (cid + 1) * NPC)
        in5 = np.ascontiguousarray(np.stack([r[s], g[s], b[s], cx[s], cy[s]]))
        inx = np.ascontiguousarray(
            np.stack([r[s], g[s], b[s], ones])).astype(BF16)
        in_maps.append({"in5": in5, "inx": inx, **stats})

    if "nc" not in _CACHE:
        _CACHE["nc"] = build_kernel()
    nc = _CACHE["nc"]
    res = run_bass_kernel_spmd(nc, in_maps, list(range(NCORES)))
    out = np.concatenate([res.results[cid]["out3"].T for cid in range(NCORES)], 0)
    return np.ascontiguousarray(out.reshape(B, H, W, 3).astype(np.float32))


# revision 13
# speedup vs baseline: 1.1324x; 1.1324x over previous
"""Bilateral grid slice+apply on 8 Trainium2 NeuronCores.

Gather-free formulation: the per-pixel trilinear interpolation is expressed
in the hat-function basis  hat(a) = relu(1 - |a|)  and evaluated densely as
matmuls with the (tiny) grid as the stationary operand:

    coeffs[n, z, c] = sum_{y,x} hy(n,y) hx(n,x) * G[y, x, z, c]     (PE, K=256)
    out[n, c3]      = sum_{z,j} hz(n,z) * xt(n,j) * coeffs[n, z, 4c3+j]

Pixels ride the matmul free dimension.  v4 layout: the PE only runs the
irreducible matmuls (hat-args, the two K=128 mains, the K=96 reduce); all
hat *replications* (hy/hx/hz fan-out to the 128/96-row product layouts) are
stride-0 SBUF->SBUF DMAs, and the xt fan-out is a stride-0 DMA straight
from DRAM.  Tiles are processed in pairs (free dim 2F=1024) to halve
instruction dispatch and DMA count; matmuls still run at N=512 per PSUM
bank.  bf16 everywhere except the coordinate path (fp32r) and PSUM (fp32).

Data parallel: pixels are sharded across the 8 cores; the 16x16x8x12 grid
is replicated (host bakes it into the stationary operands).
"""
import ml_dtypes
import numpy as np
from contextlib import ExitStack

import concourse.bass as bass
import concourse.bacc as bacc
import concourse.mybir as mybir
from concourse import tile
from concourse.bass_utils import run_bass_kernel_spmd

F = 512             # pixels per matmul pass (one fp32 PSUM bank)
NCORES = 8
B, H, W = 4, 1080, 1920
NTOT = B * H * W                  # 8294400
NPC = NTOT // NCORES              # 1036800 per core
T = NPC // F                      # 2025 tiles per core
LUM = (0.2126, 0.7152, 0.0722)

_CACHE = {}
BF16 = np.float16


def _make_stationaries(grid):
    g = grid.astype(np.float32)
    stP0 = np.zeros((5, 72), np.float32)      # rows (r,g,b,cx,cy)
    for m in range(16):
        stP0[4, m] = 15.0                     # gy from cy
    for m in range(32, 48):
        stP0[3, m] = 15.0                     # gx from cx
    for m in range(64, 72):
        stP0[0, m] = 7.0 * LUM[0]
        stP0[1, m] = 7.0 * LUM[1]
        stP0[2, m] = 7.0 * LUM[2]
    bias40 = np.zeros((72, 1), np.float32)
    bias40[:16, 0] = -np.arange(16)
    bias40[32:48, 0] = -np.arange(16)
    bias40[64:72, 0] = -np.arange(8)

    stHXa = np.zeros((16, 128), np.float32)
    for m in range(128):
        stHXa[m % 16, m] = 1.0

    stMAIN = np.zeros((2, 128, 96), np.float32)
    for p in range(2):
        for k in range(128):
            stMAIN[p, k, :] = g[p * 8 + k // 16, k % 16].reshape(96)

    stRED = np.zeros((96, 3), np.float32)
    for z in range(8):
        for c3 in range(3):
            for j in range(4):
                stRED[z * 12 + c3 * 4 + j, c3] = 1.0

    return dict(stP0=stP0, bias40=bias40, stHXa=stHXa,
                stMAINa=stMAIN[0], stMAINb=stMAIN[1], stRED=stRED)


def _cast_stationaries(stats):
    """bf16 for everything that feeds a matmul; fp32 for the Act bias and
    the fp32r coordinate path."""
    keep = ("bias40", "stP0")
    return {k: (v if k in keep else v.astype(BF16)) for k, v in stats.items()}


def make_in_map(p, c, grid):
    """One core's input map from flat p [n,3], c [n,2] and the grid."""
    n = p.shape[0]
    stats = _cast_stationaries(_make_stationaries(np.asarray(grid, np.float32)))
    in5 = np.ascontiguousarray(
        np.stack([p[:, 0], p[:, 1], p[:, 2], c[:, 0], c[:, 1]]))
    inx = np.ascontiguousarray(
        np.stack([p[:, 0], p[:, 1], p[:, 2],
                  np.ones(n, np.float32)])).astype(BF16)
    return {"in5": in5, "inx": inx, **stats}


def build_kernel(ntiles=T, num_cores=NCORES, reps=1):
    nc = bacc.Bacc("TRN2", target_bir_lowering=False, debug=False,
                   num_devices=num_cores)
    NP = ntiles * F
    f32 = mybir.dt.float32
    f32r = mybir.dt.float32r
    bf16 = mybir.dt.float16

    in5 = nc.declare_dram_parameter("in5", [5, NP], f32r, isOutput=False)
    inx = nc.declare_dram_parameter("inx", [4, NP], bf16, isOutput=False)
    decls = {}
    for nm, shp, dt_ in (("stP0", [5, 72], f32r), ("bias40", [72, 1], f32),
                         ("stHXa", [16, 128], bf16),
                         ("stMAINa", [128, 96], bf16),
                         ("stMAINb", [128, 96], bf16),
                         ("stRED", [96, 3], bf16)):
        decls[nm] = nc.declare_dram_parameter(nm, shp, dt_, isOutput=False)
    out3 = nc.declare_dram_parameter("out3", [3, NP], f32, isOutput=True)

    P2 = 2 * F
    npairs, tail = ntiles // 2, ntiles % 2

    with tile.TileContext(nc) as tc:
        with ExitStack() as ctx:
            stp = ctx.enter_context(tc.tile_pool(name="stats", bufs=1))
            sP0 = stp.tile([5, 72], f32r, tag="sP0")
            sB40 = stp.tile([72, 1], f32, tag="sB40")
            sHXa_t = stp.tile([48, 128], bf16, tag="sHXa")
            sHXa = sHXa_t[32:48, :]
            sMa = stp.tile([128, 96], bf16, tag="sMa")
            sMb = stp.tile([128, 96], bf16, tag="sMb")
            sRED = stp.tile([96, 3], bf16, tag="sRED")
            for t_, nm in ((sP0[:], "stP0"), (sB40[:], "bias40"),
                           (sHXa, "stHXa"),
                           (sMa[:], "stMAINa"), (sMb[:], "stMAINb"),
                           (sRED[:], "stRED")):
                nc.sync.dma_start(t_, decls[nm].ap())

            sb_in = ctx.enter_context(tc.tile_pool(name="sb_in", bufs=3))
            sb_mid = ctx.enter_context(tc.tile_pool(name="sb_mid", bufs=3))
            sb_rep = ctx.enter_context(tc.tile_pool(name="sb_rep", bufs=2))
            sb_w = ctx.enter_context(tc.tile_pool(name="sb_w", bufs=2))
            ps_args = ctx.enter_context(
                tc.tile_pool(name="ps_args", bufs=1, space="PSUM"))
            ps_hx = ctx.enter_context(
                tc.tile_pool(name="ps_hx", bufs=1, space="PSUM"))
            ps_cf = ctx.enter_context(
                tc.tile_pool(name="ps_cf", bufs=1, space="PSUM"))
            ps_out = ctx.enter_context(
                tc.tile_pool(name="ps_out", bufs=1, space="PSUM"))

            def mm(out, lhsT, rhs, start=True, stop=True):
                nc.tensor.matmul(out, lhsT, rhs, start=start, stop=stop)

            def pair_body(cols, c0):
                """Emit one pair (or tail) body: `cols` pixels starting at
                full-row column c0."""
                IN5 = sb_in.tile([5, P2], f32r, tag="in5p", name="IN5")
                nc.gpsimd.dma_start(IN5[:, 0:cols],
                                    in5.ap()[:, c0:c0 + cols])
                X96 = sb_in.tile([96, P2], bf16, tag="x96p", name="X96")
                nc.gpsimd.dma_start(
                    X96[:, 0:cols],
                    inx.ap()[:, c0:c0 + cols].partition_broadcast(24))

                argsP = ps_args.tile([72, P2], f32, tag="args", name="argsP")
                for h in range(0, cols, F):
                    mm(argsP[:, h:h + F], sP0[:], IN5[:, h:h + F])
                tabs = sb_mid.tile([72, P2], bf16, tag="tabs", name="tabs")
                nc.scalar.activation(tabs[:, 0:cols], argsP[:, 0:cols],
                                     mybir.ActivationFunctionType.Abs,
                                     bias=sB40[:], scale=1.0)
                hats = sb_mid.tile([72, P2], bf16, tag="hats", name="hats")
                nc.scalar.activation(hats[:, 0:cols], tabs[:, 0:cols],
                                     mybir.ActivationFunctionType.Relu,
                                     bias=1.0, scale=-1.0)

                # hat replications: stride-0 SBUF->SBUF DMAs
                HYa = sb_rep.tile([128, P2], bf16, tag="hya", name="HYa")
                HYb = sb_rep.tile([128, P2], bf16, tag="hyb", name="HYb")
                HZ = sb_rep.tile([96, P2], bf16, tag="hz", name="HZ")
                nc.sync.dma_start(
                    HYa[:, 0:cols],
                    hats[0:8, 0:cols].unsqueeze(1).broadcast_to(
                        (8, 16, cols)))
                nc.sync.dma_start(
                    HYb[:, 0:cols],
                    hats[8:16, 0:cols].unsqueeze(1).broadcast_to(
                        (8, 16, cols)))
                nc.sync.dma_start(
                    HZ[:, 0:cols],
                    hats[64:72, 0:cols].unsqueeze(1).broadcast_to(
                        (8, 12, cols)))

                HX = ps_hx.tile([128, P2], f32, tag="hx", name="HX")
                for h in range(0, cols, F):
                    mm(HX[:, h:h + F], sHXa, hats[32:48, h:h + F])
                Wa = sb_w.tile([128, P2], bf16, tag="wa", name="Wa")
                Wb = sb_w.tile([128, P2], bf16, tag="wb", name="Wb")
                HZX = sb_w.tile([96, P2], bf16, tag="hzx", name="HZX")
                nc.vector.tensor_tensor(out=Wa[:, 0:cols],
                                        in0=HYa[:, 0:cols],
                                        in1=HX[:, 0:cols],
                                        op=mybir.AluOpType.mult)
                nc.vector.tensor_tensor(out=Wb[:, 0:cols],
                                        in0=HYb[:, 0:cols],
                                        in1=HX[:, 0:cols],
                                        op=mybir.AluOpType.mult)
                nc.vector.tensor_tensor(out=HZX[:, 0:cols],
                                        in0=HZ[:, 0:cols],
                                        in1=X96[:, 0:cols],
                                        op=mybir.AluOpType.mult)

                CF = ps_cf.tile([96, P2], f32, tag="cf", name="CF")
                for h in range(0, cols, F):
                    mm(CF[:, h:h + F], sMa[:], Wa[:, h:h + F],
                       start=True, stop=False)
                    mm(CF[:, h:h + F], sMb[:], Wb[:, h:h + F],
                       start=False, stop=True)
                M2 = sb_w.tile([96, P2], bf16, tag="m2", name="M2")
                nc.vector.tensor_tensor(out=M2[:, 0:cols],
                                        in0=CF[:, 0:cols],
                                        in1=HZX[:, 0:cols],
                                        op=mybir.AluOpType.mult)
                OUT3 = ps_out.tile([3, P2], f32, tag="o3", name="OUT3")
                for h in range(0, cols, F):
                    mm(OUT3[:, h:h + F], sRED[:], M2[:, h:h + F])
                OUTS = sb_in.tile([3, P2], f32, tag="outs", name="OUTS")
                nc.scalar.copy(OUTS[:, 0:cols], OUT3[:, 0:cols])
                nc.gpsimd.dma_start(out3.ap()[:, c0:c0 + cols],
                                    OUTS[:, 0:cols])

            for _rep in range(reps):
                for pi in range(npairs):
                    pair_body(P2, pi * P2)
                if tail:
                    pair_body(F, npairs * P2)

    nc.compile()
    return nc


def kernel(pixels: np.ndarray, coords: np.ndarray, grid: np.ndarray) -> np.ndarray:
    assert pixels.shape == (B, H, W, 3) and coords.shape == (B, H, W, 2)
    p = np.asarray(pixels, np.float32).reshape(-1, 3)
    c = np.asarray(coords, np.float32).reshape(-1, 2)
    r = np.ascontiguousarray(p[:, 0]); g = np.ascontiguousarray(p[:, 1])
    b = np.ascontiguousarray(p[:, 2])
    cx = np.ascontiguousarray(c[:, 0]); cy = np.ascontiguousarray(c[:, 1])
    ones = np.ones(NPC, np.float32)

    stats = _cast_stationaries(_make_stationaries(np.asarray(grid, np.float32)))
    in_maps = []
    for cid in range(NCORES):
        s = slice(cid * NPC, (cid + 1) * NPC)
        in5 = np.ascontiguousarray(np.stack([r[s], g[s], b[s], cx[s], cy[s]]))
        inx = np.ascontiguousarray(
            np.stack([r[s], g[s], b[s], ones])).astype(BF16)
        in_maps.append({"in5": in5, "inx": inx, **stats})

    if "nc" not in _CACHE:
        _CACHE["nc"] = build_kernel()
    nc = _CACHE["nc"]
    res = run_bass_kernel_spmd(nc, in_maps, list(range(NCORES)))
    out = np.concatenate([res.results[cid]["out3"].T for cid in range(NCORES)], 0)
    return np.ascontiguousarray(out.reshape(B, H, W, 3).astype(np.float32))


# revision 14
# speedup vs baseline: 1.4363x; 1.2684x over previous
"""Bilateral grid slice+apply on 8 Trainium2 NeuronCores.

Gather-free formulation: the per-pixel trilinear interpolation is expressed
in the hat-function basis  hat(a) = relu(1 - |a|)  and evaluated densely as
matmuls with the (tiny) grid as the stationary operand:

    coeffs[n, z, c] = sum_{y,x} hy(n,y) hx(n,x) * G[y, x, z, c]     (PE, K=256)
    out[n, c3]      = sum_{z,j} hz(n,z) * xt(n,j) * coeffs[n, z, 4c3+j]

Pixels ride the matmul free dimension.  v4 layout: the PE only runs the
irreducible matmuls (hat-args, the two K=128 mains, the K=96 reduce); all
hat *replications* (hy/hx/hz fan-out to the 128/96-row product layouts) are
stride-0 SBUF->SBUF DMAs, and the xt fan-out is a stride-0 DMA straight
from DRAM.  Tiles are processed in pairs (free dim 2F=1024) to halve
instruction dispatch and DMA count; matmuls still run at N=512 per PSUM
bank.  bf16 everywhere except the coordinate path (fp32r) and PSUM (fp32).

Data parallel: pixels are sharded across the 8 cores; the 16x16x8x12 grid
is replicated (host bakes it into the stationary operands).
"""
import ml_dtypes
import numpy as np
from contextlib import ExitStack

import concourse.bass as bass
import concourse.bacc as bacc
import concourse.mybir as mybir
from concourse import tile
from concourse.bass_utils import run_bass_kernel_spmd

F = 512             # pixels per matmul pass (one fp32 PSUM bank)
NCORES = 8
B, H, W = 4, 1080, 1920
NTOT = B * H * W                  # 8294400
NPC = NTOT // NCORES              # 1036800 per core
T = NPC // F                      # 2025 tiles per core
LUM = (0.2126, 0.7152, 0.0722)

_CACHE = {}
BF16 = np.float16


def _make_stationaries(grid):
    g = grid.astype(np.float32)
    stP0 = np.zeros((5, 72), np.float32)      # rows (r,g,b,cx,cy)
    for m in range(16):
        stP0[4, m] = 15.0                     # gy from cy
    for m in range(32, 48):
        stP0[3, m] = 15.0                     # gx from cx
    for m in range(64, 72):
        stP0[0, m] = 7.0 * LUM[0]
        stP0[1, m] = 7.0 * LUM[1]
        stP0[2, m] = 7.0 * LUM[2]
    bias40 = np.zeros((72, 1), np.float32)
    bias40[:16, 0] = -np.arange(16)
    bias40[32:48, 0] = -np.arange(16)
    bias40[64:72, 0] = -np.arange(8)

    stHXa = np.zeros((16, 128), np.float32)
    for m in range(128):
        stHXa[m % 16, m] = 1.0

    stHZ = np.zeros((8, 96), np.float32)
    for z in range(8):
        stHZ[z, z * 12:(z + 1) * 12] = 1.0

    stMAIN = np.zeros((2, 128, 96), np.float32)
    for p in range(2):
        for k in range(128):
            stMAIN[p, k, :] = g[p * 8 + k // 16, k % 16].reshape(96)

    stRED = np.zeros((96, 3), np.float32)
    for z in range(8):
        for c3 in range(3):
            for j in range(4):
                stRED[z * 12 + c3 * 4 + j, c3] = 1.0

    return dict(stP0=stP0, bias40=bias40, stHXa=stHXa, stHZ=stHZ,
                stMAINa=stMAIN[0], stMAINb=stMAIN[1], stRED=stRED)


def _cast_stationaries(stats):
    """bf16 for everything that feeds a matmul; fp32 for the Act bias and
    the fp32r coordinate path."""
    keep = ("bias40", "stP0")
    return {k: (v if k in keep else v.astype(BF16)) for k, v in stats.items()}


def make_in_map(p, c, grid):
    """One core's input map from flat p [n,3], c [n,2] and the grid."""
    n = p.shape[0]
    stats = _cast_stationaries(_make_stationaries(np.asarray(grid, np.float32)))
    in5 = np.ascontiguousarray(
        np.stack([p[:, 0], p[:, 1], p[:, 2], c[:, 0], c[:, 1]]))
    inx = np.ascontiguousarray(
        np.stack([p[:, 0], p[:, 1], p[:, 2],
                  np.ones(n, np.float32)])).astype(BF16)
    return {"in5": in5, "inx": inx, **stats}


def build_kernel(ntiles=T, num_cores=NCORES, reps=1):
    nc = bacc.Bacc("TRN2", target_bir_lowering=False, debug=False,
                   num_devices=num_cores)
    NP = ntiles * F
    f32 = mybir.dt.float32
    f32r = mybir.dt.float32r
    bf16 = mybir.dt.float16

    in5 = nc.declare_dram_parameter("in5", [5, NP], f32r, isOutput=False)
    inx = nc.declare_dram_parameter("inx", [4, NP], bf16, isOutput=False)
    decls = {}
    for nm, shp, dt_ in (("stP0", [5, 72], f32r), ("bias40", [72, 1], f32),
                         ("stHXa", [16, 128], bf16),
                         ("stHZ", [8, 96], bf16),
                         ("stMAINa", [128, 96], bf16),
                         ("stMAINb", [128, 96], bf16),
                         ("stRED", [96, 3], bf16)):
        decls[nm] = nc.declare_dram_parameter(nm, shp, dt_, isOutput=False)
    out3 = nc.declare_dram_parameter("out3", [3, NP], f32, isOutput=True)

    P2 = 2 * F
    npairs, tail = ntiles // 2, ntiles % 2

    with tile.TileContext(nc) as tc:
        with ExitStack() as ctx:
            stp = ctx.enter_context(tc.tile_pool(name="stats", bufs=1))
            sP0 = stp.tile([5, 72], f32r, tag="sP0")
            sB40 = stp.tile([72, 1], f32, tag="sB40")
            sHXa_t = stp.tile([48, 128], bf16, tag="sHXa")
            sHXa = sHXa_t[32:48, :]
            sHZ_t = stp.tile([72, 96], bf16, tag="sHZ")
            sHZ = sHZ_t[64:72, :]
            sMa = stp.tile([128, 96], bf16, tag="sMa")
            sMb = stp.tile([128, 96], bf16, tag="sMb")
            sRED = stp.tile([96, 3], bf16, tag="sRED")
            for t_, nm in ((sP0[:], "stP0"), (sB40[:], "bias40"),
                           (sHXa, "stHXa"), (sHZ, "stHZ"),
                           (sMa[:], "stMAINa"), (sMb[:], "stMAINb"),
                           (sRED[:], "stRED")):
                nc.sync.dma_start(t_, decls[nm].ap())

            sb_in = ctx.enter_context(tc.tile_pool(name="sb_in", bufs=3))
            sb_mid = ctx.enter_context(tc.tile_pool(name="sb_mid", bufs=3))
            sb_rep = ctx.enter_context(tc.tile_pool(name="sb_rep", bufs=2))
            sb_w = ctx.enter_context(tc.tile_pool(name="sb_w", bufs=2))
            ps_args = ctx.enter_context(
                tc.tile_pool(name="ps_args", bufs=1, space="PSUM"))
            ps_hx = ctx.enter_context(
                tc.tile_pool(name="ps_hx", bufs=1, space="PSUM"))
            ps_cf = ctx.enter_context(
                tc.tile_pool(name="ps_cf", bufs=1, space="PSUM"))
            ps_out = ctx.enter_context(
                tc.tile_pool(name="ps_out", bufs=1, space="PSUM"))

            def mm(out, lhsT, rhs, start=True, stop=True):
                nc.tensor.matmul(out, lhsT, rhs, start=start, stop=stop)

            def pair_body(cols, c0):
                """Emit one pair (or tail) body: `cols` pixels starting at
                full-row column c0."""
                IN5 = sb_in.tile([5, P2], f32r, tag="in5p", name="IN5")
                nc.gpsimd.dma_start(IN5[:, 0:cols],
                                    in5.ap()[:, c0:c0 + cols])
                X96 = sb_in.tile([96, P2], bf16, tag="x96p", name="X96")
                nc.gpsimd.dma_start(
                    X96[:, 0:cols],
                    inx.ap()[:, c0:c0 + cols].partition_broadcast(24))

                argsP = ps_args.tile([72, P2], f32, tag="args", name="argsP")
                for h in range(0, cols, F):
                    mm(argsP[:, h:h + F], sP0[:], IN5[:, h:h + F])
                tabs = sb_mid.tile([72, P2], bf16, tag="tabs", name="tabs")
                nc.scalar.activation(tabs[:, 0:cols], argsP[:, 0:cols],
                                     mybir.ActivationFunctionType.Abs,
                                     bias=sB40[:], scale=1.0)
                hats = sb_mid.tile([72, P2], bf16, tag="hats", name="hats")
                nc.scalar.activation(hats[:, 0:cols], tabs[:, 0:cols],
                                     mybir.ActivationFunctionType.Relu,
                                     bias=1.0, scale=-1.0)

                # hat replications: stride-0 SBUF->SBUF DMAs
                HYa = sb_rep.tile([128, P2], bf16, tag="hya", name="HYa")
                HYb = sb_rep.tile([128, P2], bf16, tag="hyb", name="HYb")
                nc.sync.dma_start(
                    HYa[:, 0:cols],
                    hats[0:8, 0:cols].unsqueeze(1).broadcast_to(
                        (8, 16, cols)))
                nc.sync.dma_start(
                    HYb[:, 0:cols],
                    hats[8:16, 0:cols].unsqueeze(1).broadcast_to(
                        (8, 16, cols)))
                HX = ps_hx.tile([128, P2], f32, tag="hx", name="HX")
                for h in range(0, cols, F):
                    mm(HX[:, h:h + F], sHXa, hats[32:48, h:h + F])
                Wa = sb_w.tile([128, P2], bf16, tag="wa", name="Wa")
                Wb = sb_w.tile([128, P2], bf16, tag="wb", name="Wb")
                HZX = sb_w.tile([96, P2], bf16, tag="hzx", name="HZX")
                nc.vector.tensor_tensor(out=Wa[:, 0:cols],
                                        in0=HYa[:, 0:cols],
                                        in1=HX[:, 0:cols],
                                        op=mybir.AluOpType.mult)
                nc.vector.tensor_tensor(out=Wb[:, 0:cols],
                                        in0=HYb[:, 0:cols],
                                        in1=HX[:, 0:cols],
                                        op=mybir.AluOpType.mult)
                HZ96 = ps_out.tile([96, P2], f32, tag="o3", name="HZ96")
                for h in range(0, cols, F):
                    mm(HZ96[:, h:h + F], sHZ, hats[64:72, h:h + F])
                nc.vector.tensor_tensor(out=HZX[:, 0:cols],
                                        in0=HZ96[:, 0:cols],
                                        in1=X96[:, 0:cols],
                                        op=mybir.AluOpType.mult)

                CF = ps_cf.tile([96, P2], f32, tag="cf", name="CF")
                for h in range(0, cols, F):
                    mm(CF[:, h:h + F], sMa[:], Wa[:, h:h + F],
                       start=True, stop=False)
                    mm(CF[:, h:h + F], sMb[:], Wb[:, h:h + F],
                       start=False, stop=True)
                M2 = sb_w.tile([96, P2], bf16, tag="m2", name="M2")
                nc.vector.tensor_tensor(out=M2[:, 0:cols],
                                        in0=CF[:, 0:cols],
                                        in1=HZX[:, 0:cols],
                                        op=mybir.AluOpType.mult)
                OUT3 = ps_out.tile([3, P2], f32, tag="o3", name="OUT3")
                for h in range(0, cols, F):
                    mm(OUT3[:, h:h + F], sRED[:], M2[:, h:h + F])
                OUTS = sb_in.tile([3, P2], f32, tag="outs", name="OUTS")
                nc.scalar.copy(OUTS[:, 0:cols], OUT3[:, 0:cols])
                nc.gpsimd.dma_start(out3.ap()[:, c0:c0 + cols],
                                    OUTS[:, 0:cols])

            for _rep in range(reps):
                for pi in range(npairs):
                    pair_body(P2, pi * P2)
                if tail:
                    pair_body(F, npairs * P2)

    nc.compile()
    return nc


def kernel(pixels: np.ndarray, coords: np.ndarray, grid: np.ndarray) -> np.ndarray:
    assert pixels.shape == (B, H, W, 3) and coords.shape == (B, H, W, 2)
    p = np.asarray(pixels, np.float32).reshape(-1, 3)
    c = np.asarray(coords, np.float32).reshape(-1, 2)
    r = np.ascontiguousarray(p[:, 0]); g = np.ascontiguousarray(p[:, 1])
    b = np.ascontiguousarray(p[:, 2])
    cx = np.ascontiguousarray(c[:, 0]); cy = np.ascontiguousarray(c[:, 1])
    ones = np.ones(NPC, np.float32)

    stats = _cast_stationaries(_make_stationaries(np.asarray(grid, np.float32)))
    in_maps = []
    for cid in range(NCORES):
        s = slice(cid * NPC, (cid + 1) * NPC)
        in5 = np.ascontiguousarray(np.stack([r[s], g[s], b[s], cx[s], cy[s]]))
        inx = np.ascontiguousarray(
            np.stack([r[s], g[s], b[s], ones])).astype(BF16)
        in_maps.append({"in5": in5, "inx": inx, **stats})

    if "nc" not in _CACHE:
        _CACHE["nc"] = build_kernel()
    nc = _CACHE["nc"]
    res = run_bass_kernel_spmd(nc, in_maps, list(range(NCORES)))
    out = np.concatenate([res.results[cid]["out3"].T for cid in range(NCORES)], 0)
    return np.ascontiguousarray(out.reshape(B, H, W, 3).astype(np.float32))


# revision 15
# speedup vs baseline: 1.4660x; 1.0207x over previous
"""Bilateral grid slice+apply on 8 Trainium2 NeuronCores.

Gather-free formulation: the per-pixel trilinear interpolation is expressed
in the hat-function basis  hat(a) = relu(1 - |a|)  and evaluated densely as
matmuls with the (tiny) grid as the stationary operand:

    coeffs[n, z, c] = sum_{y,x} hy(n,y) hx(n,x) * G[y, x, z, c]     (PE, K=256)
    out[n, c3]      = sum_{z,j} hz(n,z) * xt(n,j) * coeffs[n, z, 4c3+j]

Pixels ride the matmul free dimension.  v4 layout: the PE only runs the
irreducible matmuls (hat-args, the two K=128 mains, the K=96 reduce); all
hat *replications* (hy/hx/hz fan-out to the 128/96-row product layouts) are
stride-0 SBUF->SBUF DMAs, and the xt fan-out is a stride-0 DMA straight
from DRAM.  Tiles are processed in pairs (free dim 2F=1024) to halve
instruction dispatch and DMA count; matmuls still run at N=512 per PSUM
bank.  bf16 everywhere except the coordinate path (fp32r) and PSUM (fp32).

Data parallel: pixels are sharded across the 8 cores; the 16x16x8x12 grid
is replicated (host bakes it into the stationary operands).
"""
import ml_dtypes
import numpy as np
from contextlib import ExitStack

import concourse.bass as bass
import concourse.bacc as bacc
import concourse.mybir as mybir
from concourse import tile
from concourse.bass_utils import run_bass_kernel_spmd

F = 512             # pixels per matmul pass (one fp32 PSUM bank)
NCORES = 8
B, H, W = 4, 1080, 1920
NTOT = B * H * W                  # 8294400
NPC = NTOT // NCORES              # 1036800 per core
T = NPC // F                      # 2025 tiles per core
LUM = (0.2126, 0.7152, 0.0722)

_CACHE = {}
BF16 = np.float16


def _make_stationaries(grid):
    g = grid.astype(np.float32)
    stP0 = np.zeros((5, 72), np.float32)      # rows (r,g,b,cx,cy)
    for m in range(16):
        stP0[4, m] = 15.0                     # gy from cy
    for m in range(32, 48):
        stP0[3, m] = 15.0                     # gx from cx
    for m in range(64, 72):
        stP0[0, m] = 7.0 * LUM[0]
        stP0[1, m] = 7.0 * LUM[1]
        stP0[2, m] = 7.0 * LUM[2]
    bias40 = np.zeros((72, 1), np.float32)
    bias40[:16, 0] = -np.arange(16)
    bias40[32:48, 0] = -np.arange(16)
    bias40[64:72, 0] = -np.arange(8)

    stHXa = np.zeros((16, 128), np.float32)
    for m in range(128):
        stHXa[m % 16, m] = 1.0

    stHZ = np.zeros((8, 96), np.float32)
    for z in range(8):
        stHZ[z, z * 12:(z + 1) * 12] = 1.0

    stMAIN = np.zeros((2, 128, 96), np.float32)
    for p in range(2):
        for k in range(128):
            stMAIN[p, k, :] = g[p * 8 + k // 16, k % 16].reshape(96)

    stRED = np.zeros((96, 3), np.float32)
    for z in range(8):
        for c3 in range(3):
            for j in range(4):
                stRED[z * 12 + c3 * 4 + j, c3] = 1.0

    return dict(stP0=stP0, bias40=bias40, stHXa=stHXa, stHZ=stHZ,
                stMAINa=stMAIN[0], stMAINb=stMAIN[1], stRED=stRED)


def _cast_stationaries(stats):
    """bf16 for everything that feeds a matmul; fp32 for the Act bias and
    the fp32r coordinate path."""
    keep = ("bias40", "stP0")
    return {k: (v if k in keep else v.astype(BF16)) for k, v in stats.items()}


def make_in_map(p, c, grid):
    """One core's input map from flat p [n,3], c [n,2] and the grid."""
    n = p.shape[0]
    stats = _cast_stationaries(_make_stationaries(np.asarray(grid, np.float32)))
    in5 = np.ascontiguousarray(
        np.stack([p[:, 0], p[:, 1], p[:, 2], c[:, 0], c[:, 1]]))
    inx = np.ascontiguousarray(
        np.stack([p[:, 0], p[:, 1], p[:, 2],
                  np.ones(n, np.float32)])).astype(BF16)
    return {"in5": in5, "inx": inx, **stats}


def build_kernel(ntiles=T, num_cores=NCORES, reps=1):
    nc = bacc.Bacc("TRN2", target_bir_lowering=False, debug=False,
                   num_devices=num_cores)
    NP = ntiles * F
    f32 = mybir.dt.float32
    f32r = mybir.dt.float32r
    bf16 = mybir.dt.float16

    in5 = nc.declare_dram_parameter("in5", [5, NP], f32r, isOutput=False)
    inx = nc.declare_dram_parameter("inx", [4, NP], bf16, isOutput=False)
    decls = {}
    for nm, shp, dt_ in (("stP0", [5, 72], f32r), ("bias40", [72, 1], f32),
                         ("stHXa", [16, 128], bf16),
                         ("stHZ", [8, 96], bf16),
                         ("stMAINa", [128, 96], bf16),
                         ("stMAINb", [128, 96], bf16),
                         ("stRED", [96, 3], bf16)):
        decls[nm] = nc.declare_dram_parameter(nm, shp, dt_, isOutput=False)
    out3 = nc.declare_dram_parameter("out3", [3, NP], f32, isOutput=True)

    P2 = 2 * F
    npairs, tail = ntiles // 2, ntiles % 2

    with tile.TileContext(nc) as tc:
        with ExitStack() as ctx:
            stp = ctx.enter_context(tc.tile_pool(name="stats", bufs=1))
            sP0 = stp.tile([5, 72], f32r, tag="sP0")
            sB40 = stp.tile([72, 1], f32, tag="sB40")
            sHXa_t = stp.tile([48, 128], bf16, tag="sHXa")
            sHXa = sHXa_t[32:48, :]
            sHZ_t = stp.tile([72, 96], bf16, tag="sHZ")
            sHZ = sHZ_t[64:72, :]
            sMa = stp.tile([128, 96], bf16, tag="sMa")
            sMb = stp.tile([128, 96], bf16, tag="sMb")
            sRED = stp.tile([96, 3], bf16, tag="sRED")
            for t_, nm in ((sP0[:], "stP0"), (sB40[:], "bias40"),
                           (sHXa, "stHXa"), (sHZ, "stHZ"),
                           (sMa[:], "stMAINa"), (sMb[:], "stMAINb"),
                           (sRED[:], "stRED")):
                nc.sync.dma_start(t_, decls[nm].ap())

            sb_in = ctx.enter_context(tc.tile_pool(name="sb_in", bufs=3))
            sb_mid = ctx.enter_context(tc.tile_pool(name="sb_mid", bufs=3))
            sb_rep = ctx.enter_context(tc.tile_pool(name="sb_rep", bufs=2))
            sb_w = ctx.enter_context(tc.tile_pool(name="sb_w", bufs=2))
            ps_args = ctx.enter_context(
                tc.tile_pool(name="ps_args", bufs=2, space="PSUM"))
            ps_hx = ctx.enter_context(
                tc.tile_pool(name="ps_hx", bufs=2, space="PSUM"))
            ps_cf = ctx.enter_context(
                tc.tile_pool(name="ps_cf", bufs=2, space="PSUM"))
            ps_out = ctx.enter_context(
                tc.tile_pool(name="ps_out", bufs=2, space="PSUM"))

            def mm(out, lhsT, rhs, start=True, stop=True):
                nc.tensor.matmul(out, lhsT, rhs, start=start, stop=stop)

            def pair_body(cols, c0):
                """Emit one pair (or tail) body: `cols` pixels starting at
                full-row column c0."""
                IN5 = sb_in.tile([5, P2], f32r, tag="in5p", name="IN5")
                nc.gpsimd.dma_start(IN5[:, 0:cols],
                                    in5.ap()[:, c0:c0 + cols])
                X96 = sb_in.tile([96, P2], bf16, tag="x96p", name="X96")
                nc.gpsimd.dma_start(
                    X96[:, 0:cols],
                    inx.ap()[:, c0:c0 + cols].partition_broadcast(24))

                hats = sb_mid.tile([72, P2], bf16, tag="hats", name="hats")
                for h in range(0, cols, F):
                    argsP = ps_args.tile([72, F], f32, tag="args",
                                         name="argsP")
                    mm(argsP[:], sP0[:], IN5[:, h:h + F])
                    tabs = sb_mid.tile([72, F], bf16, tag="tabs", name="tabs")
                    nc.scalar.activation(tabs[:], argsP[:],
                                         mybir.ActivationFunctionType.Abs,
                                         bias=sB40[:], scale=1.0)
                    nc.scalar.activation(hats[:, h:h + F], tabs[:],
                                         mybir.ActivationFunctionType.Relu,
                                         bias=1.0, scale=-1.0)

                # hat replications: stride-0 SBUF->SBUF DMAs
                HYa = sb_rep.tile([128, P2], bf16, tag="hya", name="HYa")
                HYb = sb_rep.tile([128, P2], bf16, tag="hyb", name="HYb")
                nc.sync.dma_start(
                    HYa[:, 0:cols],
                    hats[0:8, 0:cols].unsqueeze(1).broadcast_to(
                        (8, 16, cols)))
                nc.sync.dma_start(
                    HYb[:, 0:cols],
                    hats[8:16, 0:cols].unsqueeze(1).broadcast_to(
                        (8, 16, cols)))
                OUTS = sb_in.tile([3, P2], f32, tag="outs", name="OUTS")
                for h in range(0, cols, F):
                    HX = ps_hx.tile([128, F], f32, tag="hx", name="HX")
                    mm(HX[:], sHXa, hats[32:48, h:h + F])
                    Wa = sb_w.tile([128, F], bf16, tag="wa", name="Wa")
                    Wb = sb_w.tile([128, F], bf16, tag="wb", name="Wb")
                    HZX = sb_w.tile([96, F], bf16, tag="hzx", name="HZX")
                    nc.vector.tensor_tensor(out=Wa[:], in0=HYa[:, h:h + F],
                                            in1=HX[:],
                                            op=mybir.AluOpType.mult)
                    nc.vector.tensor_tensor(out=Wb[:], in0=HYb[:, h:h + F],
                                            in1=HX[:],
                                            op=mybir.AluOpType.mult)
                    HZ96 = ps_out.tile([96, F], f32, tag="o3", name="HZ96")
                    mm(HZ96[:], sHZ, hats[64:72, h:h + F])
                    nc.vector.tensor_tensor(out=HZX[:], in0=HZ96[:],
                                            in1=X96[:, h:h + F],
                                            op=mybir.AluOpType.mult)
                    CF = ps_cf.tile([96, F], f32, tag="cf", name="CF")
                    mm(CF[:], sMa[:], Wa[:], start=True, stop=False)
                    mm(CF[:], sMb[:], Wb[:], start=False, stop=True)
                    M2 = sb_w.tile([96, F], bf16, tag="m2", name="M2")
                    nc.vector.tensor_tensor(out=M2[:], in0=CF[:],
                                            in1=HZX[:],
                                            op=mybir.AluOpType.mult)
                    OUT3 = ps_out.tile([3, F], f32, tag="o3", name="OUT3")
                    mm(OUT3[:], sRED[:], M2[:])
                    nc.scalar.copy(OUTS[:, h:h + F], OUT3[:])
                nc.gpsimd.dma_start(out3.ap()[:, c0:c0 + cols],
                                    OUTS[:, 0:cols])

            for _rep in range(reps):
                for pi in range(npairs):
                    pair_body(P2, pi * P2)
                if tail:
                    pair_body(F, npairs * P2)

    nc.compile()
    return nc


def kernel(pixels: np.ndarray, coords: np.ndarray, grid: np.ndarray) -> np.ndarray:
    assert pixels.shape == (B, H, W, 3) and coords.shape == (B, H, W, 2)
    p = np.asarray(pixels, np.float32).reshape(-1, 3)
    c = np.asarray(coords, np.float32).reshape(-1, 2)
    r = np.ascontiguousarray(p[:, 0]); g = np.ascontiguousarray(p[:, 1])
    b = np.ascontiguousarray(p[:, 2])
    cx = np.ascontiguousarray(c[:, 0]); cy = np.ascontiguousarray(c[:, 1])
    ones = np.ones(NPC, np.float32)

    stats = _cast_stationaries(_make_stationaries(np.asarray(grid, np.float32)))
    in_maps = []
    for cid in range(NCORES):
        s = slice(cid * NPC, (cid + 1) * NPC)
        in5 = np.ascontiguousarray(np.stack([r[s], g[s], b[s], cx[s], cy[s]]))
        inx = np.ascontiguousarray(
            np.stack([r[s], g[s], b[s], ones])).astype(BF16)
        in_maps.append({"in5": in5, "inx": inx, **stats})

    if "nc" not in _CACHE:
        _CACHE["nc"] = build_kernel()
    nc = _CACHE["nc"]
    res = run_bass_kernel_spmd(nc, in_maps, list(range(NCORES)))
    out = np.concatenate([res.results[cid]["out3"].T for cid in range(NCORES)], 0)
    return np.ascontiguousarray(out.reshape(B, H, W, 3).astype(np.float32))


# revision 17
# speedup vs baseline: 1.4877x; 1.0148x over previous
"""Bilateral grid slice+apply on 8 Trainium2 NeuronCores.

Gather-free formulation: the per-pixel trilinear interpolation is expressed
in the hat-function basis  hat(a) = relu(1 - |a|)  and evaluated densely as
matmuls with the (tiny) grid as the stationary operand:

    coeffs[n, z, c] = sum_{y,x} hy(n,y) hx(n,x) * G[y, x, z, c]     (PE, K=256)
    out[n, c3]      = sum_{z,j} hz(n,z) * xt(n,j) * coeffs[n, z, 4c3+j]

Pixels ride the matmul free dimension.  v4 layout: the PE only runs the
irreducible matmuls (hat-args, the two K=128 mains, the K=96 reduce); all
hat *replications* (hy/hx/hz fan-out to the 128/96-row product layouts) are
stride-0 SBUF->SBUF DMAs, and the xt fan-out is a stride-0 DMA straight
from DRAM.  Tiles are processed in pairs (free dim 2F=1024) to halve
instruction dispatch and DMA count; matmuls still run at N=512 per PSUM
bank.  bf16 everywhere except the coordinate path (fp32r) and PSUM (fp32).

Data parallel: pixels are sharded across the 8 cores; the 16x16x8x12 grid
is replicated (host bakes it into the stationary operands).
"""
import ml_dtypes
import numpy as np
from contextlib import ExitStack

import concourse.bass as bass
import concourse.bacc as bacc
import concourse.mybir as mybir
from concourse import tile
from concourse.bass_utils import run_bass_kernel_spmd

F = 512             # pixels per matmul pass (one fp32 PSUM bank)
NCORES = 8
B, H, W = 4, 1080, 1920
NTOT = B * H * W                  # 8294400
NPC = NTOT // NCORES              # 1036800 per core
T = NPC // F                      # 2025 tiles per core
LUM = (0.2126, 0.7152, 0.0722)

_CACHE = {}
BF16 = np.float16


def _make_stationaries(grid):
    g = grid.astype(np.float32)
    stP0 = np.zeros((5, 72), np.float32)      # rows (r,g,b,cx,cy)
    for m in range(16):
        stP0[4, m] = 15.0                     # gy from cy
    for m in range(32, 48):
        stP0[3, m] = 15.0                     # gx from cx
    for m in range(64, 72):
        stP0[0, m] = 7.0 * LUM[0]
        stP0[1, m] = 7.0 * LUM[1]
        stP0[2, m] = 7.0 * LUM[2]
    bias40 = np.zeros((72, 1), np.float32)
    bias40[:16, 0] = -np.arange(16)
    bias40[32:48, 0] = -np.arange(16)
    bias40[64:72, 0] = -np.arange(8)

    stHXa = np.zeros((16, 128), np.float32)
    for m in range(128):
        stHXa[m % 16, m] = 1.0

    stHZ = np.zeros((8, 96), np.float32)
    for z in range(8):
        stHZ[z, z * 12:(z + 1) * 12] = 1.0

    stMAIN = np.zeros((2, 128, 96), np.float32)
    for p in range(2):
        for k in range(128):
            stMAIN[p, k, :] = g[p * 8 + k // 16, k % 16].reshape(96)

    stRED = np.zeros((96, 3), np.float32)
    for z in range(8):
        for c3 in range(3):
            for j in range(4):
                stRED[z * 12 + c3 * 4 + j, c3] = 1.0

    return dict(stP0=stP0, bias40=bias40, stHXa=stHXa, stHZ=stHZ,
                stMAINa=stMAIN[0], stMAINb=stMAIN[1], stRED=stRED)


def _cast_stationaries(stats):
    """bf16 for everything that feeds a matmul; fp32 for the Act bias and
    the fp32r coordinate path."""
    keep = ("bias40", "stP0")
    return {k: (v if k in keep else v.astype(BF16)) for k, v in stats.items()}


def make_in_map(p, c, grid):
    """One core's input map from flat p [n,3], c [n,2] and the grid."""
    n = p.shape[0]
    stats = _cast_stationaries(_make_stationaries(np.asarray(grid, np.float32)))
    in5 = np.ascontiguousarray(
        np.stack([p[:, 0], p[:, 1], p[:, 2], c[:, 0], c[:, 1]]))
    inx = np.ascontiguousarray(
        np.stack([p[:, 0], p[:, 1], p[:, 2],
                  np.ones(n, np.float32)])).astype(BF16)
    return {"in5": in5, "inx": inx, **stats}


def build_kernel(ntiles=T, num_cores=NCORES, reps=1):
    nc = bacc.Bacc("TRN2", target_bir_lowering=False, debug=False,
                   num_devices=num_cores)
    NP = ntiles * F
    f32 = mybir.dt.float32
    f32r = mybir.dt.float32r
    bf16 = mybir.dt.float16

    in5 = nc.declare_dram_parameter("in5", [5, NP], f32r, isOutput=False)
    inx = nc.declare_dram_parameter("inx", [4, NP], bf16, isOutput=False)
    decls = {}
    for nm, shp, dt_ in (("stP0", [5, 72], f32r), ("bias40", [72, 1], f32),
                         ("stHXa", [16, 128], bf16),
                         ("stHZ", [8, 96], bf16),
                         ("stMAINa", [128, 96], bf16),
                         ("stMAINb", [128, 96], bf16),
                         ("stRED", [96, 3], bf16)):
        decls[nm] = nc.declare_dram_parameter(nm, shp, dt_, isOutput=False)
    out3 = nc.declare_dram_parameter("out3", [3, NP], f32, isOutput=True)

    P2 = 2 * F
    npairs, tail = ntiles // 2, ntiles % 2

    with tile.TileContext(nc) as tc:
        with ExitStack() as ctx:
            stp = ctx.enter_context(tc.tile_pool(name="stats", bufs=1))
            sP0 = stp.tile([5, 72], f32r, tag="sP0")
            sB40 = stp.tile([72, 1], f32, tag="sB40")
            sHXa_t = stp.tile([48, 128], bf16, tag="sHXa")
            sHXa = sHXa_t[32:48, :]
            sHZ_t = stp.tile([72, 96], bf16, tag="sHZ")
            sHZ = sHZ_t[64:72, :]
            sMa = stp.tile([128, 96], bf16, tag="sMa")
            sMb = stp.tile([128, 96], bf16, tag="sMb")
            sRED = stp.tile([96, 3], bf16, tag="sRED")
            for t_, nm in ((sP0[:], "stP0"), (sB40[:], "bias40"),
                           (sHXa, "stHXa"), (sHZ, "stHZ"),
                           (sMa[:], "stMAINa"), (sMb[:], "stMAINb"),
                           (sRED[:], "stRED")):
                nc.sync.dma_start(t_, decls[nm].ap())

            sb_in = ctx.enter_context(tc.tile_pool(name="sb_in", bufs=4))
            sb_mid = ctx.enter_context(tc.tile_pool(name="sb_mid", bufs=3))
            sb_rep = ctx.enter_context(tc.tile_pool(name="sb_rep", bufs=3))
            sb_w = ctx.enter_context(tc.tile_pool(name="sb_w", bufs=2))
            ps_args = ctx.enter_context(
                tc.tile_pool(name="ps_args", bufs=2, space="PSUM"))
            ps_hx = ctx.enter_context(
                tc.tile_pool(name="ps_hx", bufs=2, space="PSUM"))
            ps_cf = ctx.enter_context(
                tc.tile_pool(name="ps_cf", bufs=2, space="PSUM"))
            ps_out = ctx.enter_context(
                tc.tile_pool(name="ps_out", bufs=2, space="PSUM"))

            def mm(out, lhsT, rhs, start=True, stop=True):
                nc.tensor.matmul(out, lhsT, rhs, start=start, stop=stop)

            def pair_body(cols, c0):
                """Emit one pair (or tail) body: `cols` pixels starting at
                full-row column c0."""
                IN5 = sb_in.tile([5, P2], f32r, tag="in5p", name="IN5")
                nc.gpsimd.dma_start(IN5[:, 0:cols],
                                    in5.ap()[:, c0:c0 + cols])
                X96 = sb_in.tile([96, P2], bf16, tag="x96p", name="X96")
                nc.gpsimd.dma_start(
                    X96[:, 0:cols],
                    inx.ap()[:, c0:c0 + cols].partition_broadcast(24))

                hats = sb_mid.tile([72, P2], bf16, tag="hats", name="hats")
                for h in range(0, cols, F):
                    argsP = ps_args.tile([72, F], f32, tag="args",
                                         name="argsP")
                    mm(argsP[:], sP0[:], IN5[:, h:h + F])
                    tabs = sb_mid.tile([72, F], bf16, tag="tabs", name="tabs")
                    nc.scalar.activation(tabs[:], argsP[:],
                                         mybir.ActivationFunctionType.Abs,
                                         bias=sB40[:], scale=1.0)
                    nc.scalar.activation(hats[:, h:h + F], tabs[:],
                                         mybir.ActivationFunctionType.Relu,
                                         bias=1.0, scale=-1.0)

                # hat replications: stride-0 SBUF->SBUF DMAs
                HYa = sb_rep.tile([128, P2], bf16, tag="hya", name="HYa")
                HYb = sb_rep.tile([128, P2], bf16, tag="hyb", name="HYb")
                nc.sync.dma_start(
                    HYa[:, 0:cols],
                    hats[0:8, 0:cols].unsqueeze(1).broadcast_to(
                        (8, 16, cols)))
                nc.sync.dma_start(
                    HYb[:, 0:cols],
                    hats[8:16, 0:cols].unsqueeze(1).broadcast_to(
                        (8, 16, cols)))
                OUTS = sb_in.tile([3, P2], f32, tag="outs", name="OUTS")
                for h in range(0, cols, F):
                    HX = ps_hx.tile([128, F], f32, tag="hx", name="HX")
                    mm(HX[:], sHXa, hats[32:48, h:h + F])
                    Wa = sb_w.tile([128, F], bf16, tag="wa", name="Wa")
                    Wb = sb_w.tile([128, F], bf16, tag="wb", name="Wb")
                    HZX = sb_w.tile([96, F], bf16, tag="hzx", name="HZX")
                    nc.vector.tensor_tensor(out=Wa[:], in0=HYa[:, h:h + F],
                                            in1=HX[:],
                                            op=mybir.AluOpType.mult)
                    nc.vector.tensor_tensor(out=Wb[:], in0=HYb[:, h:h + F],
                                            in1=HX[:],
                                            op=mybir.AluOpType.mult)
                    HZ96 = ps_out.tile([96, F], f32, tag="o3", name="HZ96")
                    mm(HZ96[:], sHZ, hats[64:72, h:h + F])
                    nc.vector.tensor_tensor(out=HZX[:], in0=HZ96[:],
                                            in1=X96[:, h:h + F],
                                            op=mybir.AluOpType.mult)
                    CF = ps_cf.tile([96, F], f32, tag="cf", name="CF")
                    mm(CF[:], sMa[:], Wa[:], start=True, stop=False)
                    mm(CF[:], sMb[:], Wb[:], start=False, stop=True)
                    # backend (M2 / reduce / store) runs one F-tile behind:
                    # keeps the OUTS copy from head-of-line-blocking the Act
                    # queue and lets the PE start the next tile's front while
                    # this tile's DVE chain drains
                    yield CF, HZX, OUTS, h, (h + F >= cols), c0, cols

            def backend(st):
                CF, HZX, OUTS, h, last, c0, cols = st
                M2 = sb_w.tile([96, F], bf16, tag="m2", name="M2")
                nc.vector.tensor_tensor(out=M2[:], in0=CF[:],
                                        in1=HZX[:],
                                        op=mybir.AluOpType.mult)
                OUT3 = ps_out.tile([3, F], f32, tag="o3", name="OUT3")
                mm(OUT3[:], sRED[:], M2[:])
                nc.scalar.copy(OUTS[:, h:h + F], OUT3[:])
                if last:
                    nc.gpsimd.dma_start(out3.ap()[:, c0:c0 + cols],
                                        OUTS[:, 0:cols])

            for _rep in range(reps):
                pending = None
                chunks = [(P2, pi * P2) for pi in range(npairs)]
                if tail:
                    chunks.append((F, npairs * P2))
                for cw, cc in chunks:
                    for st in pair_body(cw, cc):
                        if pending is not None:
                            backend(pending)
                        pending = st
                if pending is not None:
                    backend(pending)

    nc.compile()
    return nc


def kernel(pixels: np.ndarray, coords: np.ndarray, grid: np.ndarray) -> np.ndarray:
    assert pixels.shape == (B, H, W, 3) and coords.shape == (B, H, W, 2)
    p = np.asarray(pixels, np.float32).reshape(-1, 3)
    c = np.asarray(coords, np.float32).reshape(-1, 2)
    r = np.ascontiguousarray(p[:, 0]); g = np.ascontiguousarray(p[:, 1])
    b = np.ascontiguousarray(p[:, 2])
    cx = np.ascontiguousarray(c[:, 0]); cy = np.ascontiguousarray(c[:, 1])
    ones = np.ones(NPC, np.float32)

    stats = _cast_stationaries(_make_stationaries(np.asarray(grid, np.float32)))
    in_maps = []
    for cid in range(NCORES):
        s = slice(cid * NPC, (cid + 1) * NPC)
        in5 = np.ascontiguousarray(np.stack([r[s], g[s], b[s], cx[s], cy[s]]))
        inx = np.ascontiguousarray(
            np.stack([r[s], g[s], b[s], ones])).astype(BF16)
        in_maps.append({"in5": in5, "inx": inx, **stats})

    if "nc" not in _CACHE:
        _CACHE["nc"] = build_kernel()
    nc = _CACHE["nc"]
    res = run_bass_kernel_spmd(nc, in_maps, list(range(NCORES)))
    out = np.concatenate([res.results[cid]["out3"].T for cid in range(NCORES)], 0)
    return np.ascontiguousarray(out.reshape(B, H, W, 3).astype(np.float32))


# revision 18
# speedup vs baseline: 1.6193x; 1.0885x over previous
"""Bilateral grid slice+apply on 8 Trainium2 NeuronCores.

Gather-free formulation: the per-pixel trilinear interpolation is expressed
in the hat-function basis  hat(a) = relu(1 - |a|)  and evaluated densely as
matmuls with the (tiny) grid as the stationary operand:

    coeffs[n, z, c] = sum_{y,x} hy(n,y) hx(n,x) * G[y, x, z, c]     (PE, K=256)
    out[n, c3]      = sum_{z,j} hz(n,z) * xt(n,j) * coeffs[n, z, 4c3+j]

Pixels ride the matmul free dimension.  v4 layout: the PE only runs the
irreducible matmuls (hat-args, the two K=128 mains, the K=96 reduce); all
hat *replications* (hy/hx/hz fan-out to the 128/96-row product layouts) are
stride-0 SBUF->SBUF DMAs, and the xt fan-out is a stride-0 DMA straight
from DRAM.  Tiles are processed in pairs (free dim 2F=1024) to halve
instruction dispatch and DMA count; matmuls still run at N=512 per PSUM
bank.  bf16 everywhere except the coordinate path (fp32r) and PSUM (fp32).

Data parallel: pixels are sharded across the 8 cores; the 16x16x8x12 grid
is replicated (host bakes it into the stationary operands).
"""
import ml_dtypes
import numpy as np
from contextlib import ExitStack

import concourse.bass as bass
import concourse.bacc as bacc
import concourse.mybir as mybir
from concourse import tile
from concourse.bass_utils import run_bass_kernel_spmd

F = 512             # pixels per matmul pass (one fp32 PSUM bank)
NCORES = 8
B, H, W = 4, 1080, 1920
NTOT = B * H * W                  # 8294400
NPC = NTOT // NCORES              # 1036800 per core
T = NPC // F                      # 2025 tiles per core
LUM = (0.2126, 0.7152, 0.0722)

_CACHE = {}
BF16 = np.float16


def _make_stationaries(grid):
    g = grid.astype(np.float32)
    stP0 = np.zeros((5, 72), np.float32)      # rows (r,g,b,cx,cy)
    for m in range(16):
        stP0[4, m] = 15.0                     # gy from cy
    for m in range(32, 48):
        stP0[3, m] = 15.0                     # gx from cx
    for m in range(64, 72):
        stP0[0, m] = 7.0 * LUM[0]
        stP0[1, m] = 7.0 * LUM[1]
        stP0[2, m] = 7.0 * LUM[2]
    bias40 = np.zeros((72, 1), np.float32)
    bias40[:16, 0] = -np.arange(16)
    bias40[32:48, 0] = -np.arange(16)
    bias40[64:72, 0] = -np.arange(8)

    stHXa = np.zeros((16, 128), np.float32)
    for m in range(128):
        stHXa[m % 16, m] = 1.0

    stHZ = np.zeros((8, 96), np.float32)
    for z in range(8):
        stHZ[z, z * 12:(z + 1) * 12] = 1.0

    stMAIN = np.zeros((2, 128, 96), np.float32)
    for p in range(2):
        for k in range(128):
            stMAIN[p, k, :] = g[p * 8 + k // 16, k % 16].reshape(96)

    stRED = np.zeros((96, 3), np.float32)
    for z in range(8):
        for c3 in range(3):
            for j in range(4):
                stRED[z * 12 + c3 * 4 + j, c3] = 1.0

    return dict(stP0=stP0, bias40=bias40, stHXa=stHXa, stHZ=stHZ,
                stMAINa=stMAIN[0], stMAINb=stMAIN[1], stRED=stRED)


def _cast_stationaries(stats):
    """bf16 for everything that feeds a matmul; fp32 for the Act bias and
    the fp32r coordinate path."""
    keep = ("bias40", "stP0")
    return {k: (v if k in keep else v.astype(BF16)) for k, v in stats.items()}


def make_in_map(p, c, grid):
    """One core's input map from flat p [n,3], c [n,2] and the grid."""
    n = p.shape[0]
    stats = _cast_stationaries(_make_stationaries(np.asarray(grid, np.float32)))
    in5 = np.ascontiguousarray(
        np.stack([p[:, 0], p[:, 1], p[:, 2], c[:, 0], c[:, 1]]))
    inx = np.ascontiguousarray(
        np.stack([p[:, 0], p[:, 1], p[:, 2],
                  np.ones(n, np.float32)])).astype(BF16)
    return {"in5": in5, "inx": inx, **stats}


def build_kernel(ntiles=T, num_cores=NCORES, reps=1):
    nc = bacc.Bacc("TRN2", target_bir_lowering=False, debug=False,
                   num_devices=num_cores)
    NP = ntiles * F
    f32 = mybir.dt.float32
    f32r = mybir.dt.float32r
    bf16 = mybir.dt.float16

    in5 = nc.declare_dram_parameter("in5", [5, NP], f32r, isOutput=False)
    inx = nc.declare_dram_parameter("inx", [4, NP], bf16, isOutput=False)
    decls = {}
    for nm, shp, dt_ in (("stP0", [5, 72], f32r), ("bias40", [72, 1], f32),
                         ("stHXa", [16, 128], bf16),
                         ("stHZ", [8, 96], bf16),
                         ("stMAINa", [128, 96], bf16),
                         ("stMAINb", [128, 96], bf16),
                         ("stRED", [96, 3], bf16)):
        decls[nm] = nc.declare_dram_parameter(nm, shp, dt_, isOutput=False)
    out3 = nc.declare_dram_parameter("out3", [3, NP], f32, isOutput=True)

    P2 = 2 * F
    npairs, tail = ntiles // 2, ntiles % 2

    with tile.TileContext(nc) as tc:
        with ExitStack() as ctx:
            stp = ctx.enter_context(tc.tile_pool(name="stats", bufs=1))
            sP0 = stp.tile([5, 72], f32r, tag="sP0")
            sB40 = stp.tile([72, 1], f32, tag="sB40")
            sHXa_t = stp.tile([48, 128], bf16, tag="sHXa")
            sHXa = sHXa_t[32:48, :]
            sHZ_t = stp.tile([72, 96], bf16, tag="sHZ")
            sHZ = sHZ_t[64:72, :]
            sMa = stp.tile([128, 96], bf16, tag="sMa")
            sMb = stp.tile([128, 96], bf16, tag="sMb")
            sRED = stp.tile([96, 3], bf16, tag="sRED")
            for t_, nm in ((sP0[:], "stP0"), (sB40[:], "bias40"),
                           (sHXa, "stHXa"), (sHZ, "stHZ"),
                           (sMa[:], "stMAINa"), (sMb[:], "stMAINb"),
                           (sRED[:], "stRED")):
                nc.sync.dma_start(t_, decls[nm].ap())

            sb_in = ctx.enter_context(tc.tile_pool(name="sb_in", bufs=4))
            sb_mid = ctx.enter_context(tc.tile_pool(name="sb_mid", bufs=3))
            sb_rep = ctx.enter_context(tc.tile_pool(name="sb_rep", bufs=3))
            sb_w = ctx.enter_context(tc.tile_pool(name="sb_w", bufs=2))
            ps_args = ctx.enter_context(
                tc.tile_pool(name="ps_args", bufs=2, space="PSUM"))
            ps_hx = ctx.enter_context(
                tc.tile_pool(name="ps_hx", bufs=2, space="PSUM"))
            ps_cf = ctx.enter_context(
                tc.tile_pool(name="ps_cf", bufs=2, space="PSUM"))
            ps_out = ctx.enter_context(
                tc.tile_pool(name="ps_out", bufs=2, space="PSUM"))

            def mm(out, lhsT, rhs, start=True, stop=True):
                nc.tensor.matmul(out, lhsT, rhs, start=start, stop=stop)

            def pair_body(cols, c0):
                """Emit one pair (or tail) body: `cols` pixels starting at
                full-row column c0."""
                IN5 = sb_in.tile([5, P2], f32r, tag="in5p", name="IN5")
                nc.gpsimd.dma_start(IN5[:, 0:cols],
                                    in5.ap()[:, c0:c0 + cols])
                X96 = sb_in.tile([96, P2], bf16, tag="x96p", name="X96")
                nc.gpsimd.dma_start(
                    X96[:, 0:cols],
                    inx.ap()[:, c0:c0 + cols].partition_broadcast(24))

                hats = sb_mid.tile([72, P2], bf16, tag="hats", name="hats")
                for h in range(0, cols, F):
                    argsP = ps_args.tile([72, F], f32, tag="args",
                                         name="argsP")
                    mm(argsP[:], sP0[:], IN5[:, h:h + F])
                    tabs = sb_mid.tile([72, F], bf16, tag="tabs", name="tabs")
                    nc.scalar.activation(tabs[:], argsP[:],
                                         mybir.ActivationFunctionType.Abs,
                                         bias=sB40[:], scale=1.0)
                    nc.scalar.activation(hats[:, h:h + F], tabs[:],
                                         mybir.ActivationFunctionType.Relu,
                                         bias=1.0, scale=-1.0)

                # hat replications: stride-0 SBUF->SBUF DMAs
                HYa = sb_rep.tile([128, P2], bf16, tag="hya", name="HYa")
                HYb = sb_rep.tile([128, P2], bf16, tag="hyb", name="HYb")
                nc.sync.dma_start(
                    HYa[:, 0:cols],
                    hats[0:8, 0:cols].unsqueeze(1).broadcast_to(
                        (8, 16, cols)))
                nc.sync.dma_start(
                    HYb[:, 0:cols],
                    hats[8:16, 0:cols].unsqueeze(1).broadcast_to(
                        (8, 16, cols)))
                OUTS = sb_in.tile([3, P2], f32, tag="outs", name="OUTS")
                for h in range(0, cols, F):
                    HX = ps_hx.tile([128, F], f32, tag="hx", name="HX")
                    mm(HX[:], sHXa, hats[32:48, h:h + F])
                    Wa = sb_w.tile([128, F], bf16, tag="wa", name="Wa")
                    Wb = sb_w.tile([128, F], bf16, tag="wb", name="Wb")
                    HZX = sb_w.tile([96, F], bf16, tag="hzx", name="HZX")
                    nc.vector.tensor_tensor(out=Wa[:], in0=HYa[:, h:h + F],
                                            in1=HX[:],
                                            op=mybir.AluOpType.mult)
                    nc.vector.tensor_tensor(out=Wb[:], in0=HYb[:, h:h + F],
                                            in1=HX[:],
                                            op=mybir.AluOpType.mult)
                    HZ96 = ps_out.tile([96, F], f32, tag="o3", name="HZ96")
                    mm(HZ96[:], sHZ, hats[64:72, h:h + F])
                    nc.vector.tensor_tensor(out=HZX[:], in0=HZ96[:],
                                            in1=X96[:, h:h + F],
                                            op=mybir.AluOpType.mult)
                    CF = ps_cf.tile([96, F], f32, tag="cf", name="CF")
                    mm(CF[:], sMa[:], Wa[:], start=True, stop=False)
                    mm(CF[:], sMb[:], Wb[:], start=False, stop=True)
                    # backend (M2 / reduce / store) runs one F-tile behind:
                    # keeps the OUTS copy from head-of-line-blocking the Act
                    # queue and lets the PE start the next tile's front while
                    # this tile's DVE chain drains
                    yield CF, HZX, OUTS, h, (h + F >= cols), c0, cols

            def backend(st):
                CF, HZX, OUTS, h, last, c0, cols = st
                M2 = sb_w.tile([96, F], bf16, tag="m2", name="M2")
                nc.vector.tensor_tensor(out=M2[:], in0=CF[:],
                                        in1=HZX[:],
                                        op=mybir.AluOpType.mult)
                OUT3 = ps_out.tile([3, F], f32, tag="o3", name="OUT3")
                mm(OUT3[:], sRED[:], M2[:])
                nc.scalar.copy(OUTS[:, h:h + F], OUT3[:])
                if last:
                    nc.scalar.dma_start(out3.ap()[:, c0:c0 + cols],
                                        OUTS[:, 0:cols])

            for _rep in range(reps):
                pending = None
                chunks = [(P2, pi * P2) for pi in range(npairs)]
                if tail:
                    chunks.append((F, npairs * P2))
                for cw, cc in chunks:
                    for st in pair_body(cw, cc):
                        if pending is not None:
                            backend(pending)
                        pending = st
                if pending is not None:
                    backend(pending)

    nc.compile()
    return nc


def kernel(pixels: np.ndarray, coords: np.ndarray, grid: np.ndarray) -> np.ndarray:
    assert pixels.shape == (B, H, W, 3) and coords.shape == (B, H, W, 2)
    p = np.asarray(pixels, np.float32).reshape(-1, 3)
    c = np.asarray(coords, np.float32).reshape(-1, 2)
    r = np.ascontiguousarray(p[:, 0]); g = np.ascontiguousarray(p[:, 1])
    b = np.ascontiguousarray(p[:, 2])
    cx = np.ascontiguousarray(c[:, 0]); cy = np.ascontiguousarray(c[:, 1])
    ones = np.ones(NPC, np.float32)

    stats = _cast_stationaries(_make_stationaries(np.asarray(grid, np.float32)))
    in_maps = []
    for cid in range(NCORES):
        s = slice(cid * NPC, (cid + 1) * NPC)
        in5 = np.ascontiguousarray(np.stack([r[s], g[s], b[s], cx[s], cy[s]]))
        inx = np.ascontiguousarray(
            np.stack([r[s], g[s], b[s], ones])).astype(BF16)
        in_maps.append({"in5": in5, "inx": inx, **stats})

    if "nc" not in _CACHE:
        _CACHE["nc"] = build_kernel()
    nc = _CACHE["nc"]
    res = run_bass_kernel_spmd(nc, in_maps, list(range(NCORES)))
    out = np.concatenate([res.results[cid]["out3"].T for cid in range(NCORES)], 0)
    return np.ascontiguousarray(out.reshape(B, H, W, 3).astype(np.float32))


# revision 19
# speedup vs baseline: 1.6824x; 1.0390x over previous
"""Bilateral grid slice+apply on 8 Trainium2 NeuronCores.

Gather-free formulation: the per-pixel trilinear interpolation is expressed
in the hat-function basis  hat(a) = relu(1 - |a|)  and evaluated densely as
matmuls with the (tiny) grid as the stationary operand:

    coeffs[n, z, c] = sum_{y,x} hy(n,y) hx(n,x) * G[y, x, z, c]     (PE, K=256)
    out[n, c3]      = sum_{z,j} hz(n,z) * xt(n,j) * coeffs[n, z, 4c3+j]

Pixels ride the matmul free dimension.  v4 layout: the PE only runs the
irreducible matmuls (hat-args, the two K=128 mains, the K=96 reduce); all
hat *replications* (hy/hx/hz fan-out to the 128/96-row product layouts) are
stride-0 SBUF->SBUF DMAs, and the xt fan-out is a stride-0 DMA straight
from DRAM.  Tiles are processed in pairs (free dim 2F=1024) to halve
instruction dispatch and DMA count; matmuls still run at N=512 per PSUM
bank.  bf16 everywhere except the coordinate path (fp32r) and PSUM (fp32).

Data parallel: pixels are sharded across the 8 cores; the 16x16x8x12 grid
is replicated (host bakes it into the stationary operands).
"""
import ml_dtypes
import numpy as np
from contextlib import ExitStack

import concourse.bass as bass
import concourse.bacc as bacc
import concourse.mybir as mybir
from concourse import tile
from concourse.bass_utils import run_bass_kernel_spmd

F = 512             # pixels per matmul pass (one fp32 PSUM bank)
NCORES = 8
B, H, W = 4, 1080, 1920
NTOT = B * H * W                  # 8294400
NPC = NTOT // NCORES              # 1036800 per core
T = NPC // F                      # 2025 tiles per core
LUM = (0.2126, 0.7152, 0.0722)

_CACHE = {}
BF16 = np.float16


def _make_stationaries(grid):
    g = grid.astype(np.float32)
    stP0 = np.zeros((5, 72), np.float32)      # rows (r,g,b,cx,cy)
    for m in range(16):
        stP0[4, m] = 15.0                     # gy from cy
    for m in range(32, 48):
        stP0[3, m] = 15.0                     # gx from cx
    for m in range(64, 72):
        stP0[0, m] = 7.0 * LUM[0]
        stP0[1, m] = 7.0 * LUM[1]
        stP0[2, m] = 7.0 * LUM[2]
    bias40 = np.zeros((72, 1), np.float32)
    bias40[:16, 0] = -np.arange(16)
    bias40[32:48, 0] = -np.arange(16)
    bias40[64:72, 0] = -np.arange(8)

    stHXa = np.zeros((16, 128), np.float32)
    for m in range(128):
        stHXa[m % 16, m] = 1.0

    stHZ = np.zeros((8, 96), np.float32)
    for z in range(8):
        stHZ[z, z * 12:(z + 1) * 12] = 1.0

    stMAIN = np.zeros((2, 128, 96), np.float32)
    for p in range(2):
        for k in range(128):
            stMAIN[p, k, :] = g[p * 8 + k // 16, k % 16].reshape(96)

    stRED = np.zeros((96, 3), np.float32)
    for z in range(8):
        for c3 in range(3):
            for j in range(4):
                stRED[z * 12 + c3 * 4 + j, c3] = 1.0

    return dict(stP0=stP0, bias40=bias40, stHXa=stHXa, stHZ=stHZ,
                stMAINa=stMAIN[0], stMAINb=stMAIN[1], stRED=stRED)


def _cast_stationaries(stats):
    """bf16 for everything that feeds a matmul; fp32 for the Act bias and
    the fp32r coordinate path."""
    keep = ("bias40", "stP0")
    return {k: (v if k in keep else v.astype(BF16)) for k, v in stats.items()}


def make_in_map(p, c, grid):
    """One core's input map from flat p [n,3], c [n,2] and the grid."""
    n = p.shape[0]
    stats = _cast_stationaries(_make_stationaries(np.asarray(grid, np.float32)))
    in5 = np.ascontiguousarray(
        np.stack([p[:, 0], p[:, 1], p[:, 2], c[:, 0], c[:, 1]]))
    inx = np.ascontiguousarray(
        np.stack([p[:, 0], p[:, 1], p[:, 2],
                  np.ones(n, np.float32)])).astype(BF16)
    return {"in5": in5, "inx": inx, **stats}


def build_kernel(ntiles=T, num_cores=NCORES, reps=1):
    nc = bacc.Bacc("TRN2", target_bir_lowering=False, debug=False,
                   num_devices=num_cores)
    NP = ntiles * F
    f32 = mybir.dt.float32
    f32r = mybir.dt.float32r
    bf16 = mybir.dt.float16

    in5 = nc.declare_dram_parameter("in5", [5, NP], f32r, isOutput=False)
    inx = nc.declare_dram_parameter("inx", [4, NP], bf16, isOutput=False)
    decls = {}
    for nm, shp, dt_ in (("stP0", [5, 72], f32r), ("bias40", [72, 1], f32),
                         ("stHXa", [16, 128], bf16),
                         ("stHZ", [8, 96], bf16),
                         ("stMAINa", [128, 96], bf16),
                         ("stMAINb", [128, 96], bf16),
                         ("stRED", [96, 3], bf16)):
        decls[nm] = nc.declare_dram_parameter(nm, shp, dt_, isOutput=False)
    out3 = nc.declare_dram_parameter("out3", [3, NP], f32, isOutput=True)

    P2 = 2 * F
    npairs, tail = ntiles // 2, ntiles % 2

    with tile.TileContext(nc) as tc:
        with ExitStack() as ctx:
            stp = ctx.enter_context(tc.tile_pool(name="stats", bufs=1))
            sP0 = stp.tile([5, 72], f32r, tag="sP0")
            sB40 = stp.tile([72, 1], f32, tag="sB40")
            sHXa_t = stp.tile([48, 128], bf16, tag="sHXa")
            sHXa = sHXa_t[32:48, :]
            sHZ_t = stp.tile([72, 96], bf16, tag="sHZ")
            sHZ = sHZ_t[64:72, :]
            sMa = stp.tile([128, 96], bf16, tag="sMa")
            sMb = stp.tile([128, 96], bf16, tag="sMb")
            sRED = stp.tile([96, 3], bf16, tag="sRED")
            for t_, nm in ((sP0[:], "stP0"), (sB40[:], "bias40"),
                           (sHXa, "stHXa"), (sHZ, "stHZ"),
                           (sMa[:], "stMAINa"), (sMb[:], "stMAINb"),
                           (sRED[:], "stRED")):
                nc.sync.dma_start(t_, decls[nm].ap())

            sb_in = ctx.enter_context(tc.tile_pool(name="sb_in", bufs=4))
            sb_mid = ctx.enter_context(tc.tile_pool(name="sb_mid", bufs=3))
            sb_rep = ctx.enter_context(tc.tile_pool(name="sb_rep", bufs=3))
            sb_w = ctx.enter_context(tc.tile_pool(name="sb_w", bufs=2))
            ps_args = ctx.enter_context(
                tc.tile_pool(name="ps_args", bufs=2, space="PSUM"))
            ps_hx = ctx.enter_context(
                tc.tile_pool(name="ps_hx", bufs=2, space="PSUM"))
            ps_cf = ctx.enter_context(
                tc.tile_pool(name="ps_cf", bufs=2, space="PSUM"))
            ps_out = ctx.enter_context(
                tc.tile_pool(name="ps_out", bufs=2, space="PSUM"))

            def mm(out, lhsT, rhs, start=True, stop=True):
                nc.tensor.matmul(out, lhsT, rhs, start=start, stop=stop)

            def pair_body(cols, c0):
                """Emit one pair (or tail) body: `cols` pixels starting at
                full-row column c0."""
                IN5 = sb_in.tile([5, P2], f32r, tag="in5p", name="IN5")
                nc.gpsimd.dma_start(IN5[:, 0:cols],
                                    in5.ap()[:, c0:c0 + cols])
                X96 = sb_in.tile([96, P2], bf16, tag="x96p", name="X96")
                nc.gpsimd.dma_start(
                    X96[:, 0:cols],
                    inx.ap()[:, c0:c0 + cols].partition_broadcast(24))

                hats = sb_mid.tile([72, P2], bf16, tag="hats", name="hats")
                for h in range(0, cols, F):
                    argsP = ps_args.tile([72, F], f32, tag="args",
                                         name="argsP")
                    mm(argsP[:], sP0[:], IN5[:, h:h + F])
                    tabs = sb_mid.tile([72, F], bf16, tag="tabs", name="tabs")
                    nc.scalar.activation(tabs[:], argsP[:],
                                         mybir.ActivationFunctionType.Abs,
                                         bias=sB40[:], scale=1.0)
                    nc.scalar.activation(hats[:, h:h + F], tabs[:],
                                         mybir.ActivationFunctionType.Relu,
                                         bias=1.0, scale=-1.0)

                # hat replications: stride-0 SBUF->SBUF DMAs
                HYa = sb_rep.tile([128, P2], bf16, tag="hya", name="HYa")
                HYb = sb_rep.tile([128, P2], bf16, tag="hyb", name="HYb")
                nc.sync.dma_start(
                    HYa[:, 0:cols],
                    hats[0:8, 0:cols].unsqueeze(1).broadcast_to(
                        (8, 16, cols)))
                nc.sync.dma_start(
                    HYb[:, 0:cols],
                    hats[8:16, 0:cols].unsqueeze(1).broadcast_to(
                        (8, 16, cols)))
                OUTS = sb_in.tile([3, P2], f32, tag="outs", name="OUTS")
                # stationary-major emission: both F-halves back-to-back per
                # stationary so the PE re-uses/overlaps each weight load
                hs = list(range(0, cols, F))
                HXs, HZ96s, Was, Wbs, HZXs, CFs = {}, {}, {}, {}, {}, {}
                for h in hs:
                    HXs[h] = ps_hx.tile([128, F], f32, tag="hx", name="HX")
                    mm(HXs[h][:], sHXa, hats[32:48, h:h + F])
                for h in hs:
                    HZ96s[h] = ps_out.tile([96, F], f32, tag="o3",
                                           name="HZ96")
                    mm(HZ96s[h][:], sHZ, hats[64:72, h:h + F])
                for h in hs:
                    Was[h] = sb_w.tile([128, F], bf16, tag="wa", name="Wa")
                    Wbs[h] = sb_w.tile([128, F], bf16, tag="wb", name="Wb")
                    nc.vector.tensor_tensor(out=Was[h][:],
                                            in0=HYa[:, h:h + F],
                                            in1=HXs[h][:],
                                            op=mybir.AluOpType.mult)
                    nc.vector.tensor_tensor(out=Wbs[h][:],
                                            in0=HYb[:, h:h + F],
                                            in1=HXs[h][:],
                                            op=mybir.AluOpType.mult)
                    HZXs[h] = sb_w.tile([96, F], bf16, tag="hzx",
                                        name="HZX")
                    nc.vector.tensor_tensor(out=HZXs[h][:],
                                            in0=HZ96s[h][:],
                                            in1=X96[:, h:h + F],
                                            op=mybir.AluOpType.mult)
                for h in hs:
                    CFs[h] = ps_cf.tile([96, F], f32, tag="cf", name="CF")
                    mm(CFs[h][:], sMa[:], Was[h][:], start=True, stop=False)
                    mm(CFs[h][:], sMb[:], Wbs[h][:], start=False, stop=True)
                for h in hs:
                    yield (CFs[h], HZXs[h], OUTS, h, (h + F >= cols),
                           c0, cols)

            def backend(st):
                CF, HZX, OUTS, h, last, c0, cols = st
                M2 = sb_w.tile([96, F], bf16, tag="m2", name="M2")
                nc.vector.tensor_tensor(out=M2[:], in0=CF[:],
                                        in1=HZX[:],
                                        op=mybir.AluOpType.mult)
                OUT3 = ps_out.tile([3, F], f32, tag="o3", name="OUT3")
                mm(OUT3[:], sRED[:], M2[:])
                nc.scalar.copy(OUTS[:, h:h + F], OUT3[:])
                if last:
                    nc.scalar.dma_start(out3.ap()[:, c0:c0 + cols],
                                        OUTS[:, 0:cols])

            for _rep in range(reps):
                pending = None
                chunks = [(P2, pi * P2) for pi in range(npairs)]
                if tail:
                    chunks.append((F, npairs * P2))
                for cw, cc in chunks:
                    for st in pair_body(cw, cc):
                        if pending is not None:
                            backend(pending)
                        pending = st
                if pending is not None:
                    backend(pending)

    nc.compile()
    return nc


def kernel(pixels: np.ndarray, coords: np.ndarray, grid: np.ndarray) -> np.ndarray:
    assert pixels.shape == (B, H, W, 3) and coords.shape == (B, H, W, 2)
    p = np.asarray(pixels, np.float32).reshape(-1, 3)
    c = np.asarray(coords, np.float32).reshape(-1, 2)
    r = np.ascontiguousarray(p[:, 0]); g = np.ascontiguousarray(p[:, 1])
    b = np.ascontiguousarray(p[:, 2])
    cx = np.ascontiguousarray(c[:, 0]); cy = np.ascontiguousarray(c[:, 1])
    ones = np.ones(NPC, np.float32)

    stats = _cast_stationaries(_make_stationaries(np.asarray(grid, np.float32)))
    in_maps = []
    for cid in range(NCORES):
        s = slice(cid * NPC, (cid + 1) * NPC)
        in5 = np.ascontiguousarray(np.stack([r[s], g[s], b[s], cx[s], cy[s]]))
        inx = np.ascontiguousarray(
            np.stack([r[s], g[s], b[s], ones])).astype(BF16)
        in_maps.append({"in5": in5, "inx": inx, **stats})

    if "nc" not in _CACHE:
        _CACHE["nc"] = build_kernel()
    nc = _CACHE["nc"]
    res = run_bass_kernel_spmd(nc, in_maps, list(range(NCORES)))
    out = np.concatenate([res.results[cid]["out3"].T for cid in range(NCORES)], 0)
    return np.ascontiguousarray(out.reshape(B, H, W, 3).astype(np.float32))
